# revision 1
# baseline (speedup 1.0000x reference)
"""Trainium2 Bass/Tile kernel for CrossChannelInterp.

Full computation (per batch, x split into x0/x1/x2 of (D, T) each):
    E   = exp(x1)                                  -> intensity output
    S[t] = sum_c E[c, t]                           (softmax denominator)
    mean[c] = mean_t x0[c, t]
    A   = E * (x0 - mean)                          (unnormalized sm*(y-mean))
    M   = W^T @ A                                  (d_out x T)
    rep1 = M * (1/S)[t] + mean[c]                  -> output channel block 0
    y_trans = x2 - rep1                            -> output channel block 2

Sharding: data-parallel over batch, 32 batches -> 8 cores x 4 batches.
All compute stays in the natural (channel, T) layout: channel on SBUF
partitions (4 tiles of 128), T on the free axis.
"""

import os
import sys

for _p in ("/opt/trn_rl_repo", "/root/.axon_site/_ro/trn_rl_repo"):
    if os.path.isdir(_p) and _p not in sys.path:
        sys.path.append(_p)

import numpy as np

P = 128          # SBUF partitions
D = 512          # channel dim
T = 2048         # time dim
NB = 4           # batches per core
KT = D // P      # 4 channel tiles
NCORES = 8
TCH = 512        # matmul free-dim chunk (PSUM bank)
NCHUNK = T // TCH  # 4

_cache = {}


def _build_nc(loop_iters=None, paired_stores=False):
    from contextlib import ExitStack

    import concourse.bacc as bacc
    import concourse.tile as tile
    from concourse import mybir

    f32 = mybir.dt.float32
    bf16 = mybir.dt.bfloat16
    Alu = mybir.AluOpType
    Act = mybir.ActivationFunctionType
    Axis = mybir.AxisListType

    nc = bacc.Bacc("TRN2", target_bir_lowering=False, debug=False)
    x = nc.declare_dram_parameter("x", [NB, 3 * D, T], f32, isOutput=False)
    Wp = nc.declare_dram_parameter("W", [D, D], f32, isOutput=False)
    out = nc.declare_dram_parameter("out", [NB, 3 * D, T], f32, isOutput=True)

    with ExitStack() as ctx:
        tc = ctx.enter_context(tile.TileContext(nc))

        singles = ctx.enter_context(tc.tile_pool(name="singles", bufs=1))
        px1 = ctx.enter_context(tc.tile_pool(name="px1", bufs=2))
        pE = ctx.enter_context(tc.tile_pool(name="pE", bufs=4))
        px0 = ctx.enter_context(tc.tile_pool(name="px0", bufs=2))
        pA = ctx.enter_context(tc.tile_pool(name="pA", bufs=6))
        pmean = ctx.enter_context(tc.tile_pool(name="pmean", bufs=8))
        pRb = ctx.enter_context(tc.tile_pool(name="pRb", bufs=2))
        px2 = ctx.enter_context(tc.tile_pool(name="px2", bufs=2))
        pout0 = ctx.enter_context(tc.tile_pool(name="pout0", bufs=2))
        pout2 = ctx.enter_context(tc.tile_pool(name="pout2", bufs=2))
        # PSUM: colsum-broadcast (128,512)=1 bank x4, matmul (128,1024)=2
        # banks x2  -> 8 banks total
        pS = ctx.enter_context(tc.tile_pool(name="pS", bufs=4, space="PSUM"))
        pM = ctx.enter_context(tc.tile_pool(name="pM", bufs=2, space="PSUM"))

        # --- constants ---
        # W as 4 k-tiles of (128, 512) cast to bf16 for full-rate matmul;
        # lhsT slice [:, co*128:(co+1)*128]
        w_tiles = []
        for k in range(KT):
            w_f32 = singles.tile([P, D], f32, name=f"wf_{k}")
            nc.sync.dma_start(out=w_f32, in_=Wp[k * P:(k + 1) * P, :])
            w_k = singles.tile([P, D], bf16, name=f"w_{k}")
            nc.vector.tensor_copy(w_k, w_f32)
            w_tiles.append(w_k)
        # ones (128,128): colsum matmul replicates S over all 128 output
        # partitions, giving the free-axis broadcast of 1/S for free
        ones_mat = singles.tile([P, P], f32, name="ones_mat")
        nc.vector.memset(ones_mat, 1.0)

        # benchmark mode: repeat the whole body loop_iters times at runtime;
        # hint_engines arms the branch prefetcher for the large PE body so
        # the back-edge doesn't stall on IRAM refetch
        if loop_iters is not None:
            loop_cm = tc.For_i(
                0, loop_iters, 1, hint_engines=(mybir.EngineType.PE,)
            )
            ctx.enter_context(loop_cm)

        for b in range(NB):
            # ---------- phase 1: E, colsum, mean, A ----------
            Sb_tiles = [
                pS.tile([P, TCH], f32, name=f"Sb_{tch}", tag="Sb")
                for tch in range(NCHUNK)
            ]
            A_tiles = []
            mean_tiles = []
            for k in range(KT):
                x1_k = px1.tile([P, T], f32, name="x1_k", tag="x1")
                nc.sync.dma_start(out=x1_k, in_=x[b, D + k * P:D + (k + 1) * P, :])
                E_k = pE.tile([P, T], f32, name="E_k", tag="E")
                nc.scalar.activation(out=E_k, in_=x1_k, func=Act.Exp)
                # intensity output = exp(x1); stores issue from scalar/gpsimd
                # queues to keep the sync sequencer free for loads
                nc.scalar.dma_start(out=out[b, D + k * P:D + (k + 1) * P, :], in_=E_k)
                # colsum accumulation, replicated across all 128 partitions:
                # Sb[tch][p, t] += sum_k E_k[k, t]
                for tch in range(NCHUNK):
                    nc.tensor.matmul(
                        Sb_tiles[tch],
                        lhsT=ones_mat,
                        rhs=E_k[:, tch * TCH:(tch + 1) * TCH],
                        start=(k == 0),
                        stop=(k == KT - 1),
                    )
                x0_k = px0.tile([P, T], f32, name="x0_k", tag="x0")
                nc.sync.dma_start(out=x0_k, in_=x[b, k * P:(k + 1) * P, :])
                mean_k = pmean.tile([P, 1], f32, name="mean_k", tag="mean")
                nc.vector.tensor_reduce(out=mean_k, in_=x0_k, axis=Axis.X, op=Alu.add)
                nc.vector.tensor_scalar_mul(mean_k, mean_k, 1.0 / T)
                A_k = pA.tile([P, T], bf16, name="A_k", tag="A")
                nc.vector.scalar_tensor_tensor(
                    out=A_k, in0=x0_k, scalar=mean_k, in1=E_k,
                    op0=Alu.subtract, op1=Alu.mult,
                )
                A_tiles.append(A_k)
                mean_tiles.append(mean_k)

            # ---------- phase 2: Rb = 1/S, already partition-replicated ----------
            Rb = pRb.tile([P, T], f32, name="Rb", tag="Rb")
            for tch in range(NCHUNK):
                nc.vector.reciprocal(
                    out=Rb[:, tch * TCH:(tch + 1) * TCH], in_=Sb_tiles[tch]
                )

            # ---------- phase 3: matmul + epilogue ----------
            for ci in range(2):
                if paired_stores:
                    # (128, 2, 2048) channel-pair tiles -> one 2MB store per
                    # pair instead of two 1MB stores
                    o0p = pout0.tile([P, 2, T], f32, name="o0p", tag="o0", bufs=1)
                    o2p = pout2.tile([P, 2, T], f32, name="o2p", tag="o2", bufs=1)
                for cj in range(2):
                    co = 2 * ci + cj
                    if paired_stores:
                        out0 = o0p[:, cj, :]
                        out2 = o2p[:, cj, :]
                    else:
                        out0 = pout0.tile([P, T], f32, name="out0", tag="o0")
                        out2 = pout2.tile([P, T], f32, name="out2", tag="o2")
                    x2_c = px2.tile([P, T], f32, name="x2_c", tag="x2")
                    nc.sync.dma_start(out=x2_c, in_=x[b, 2 * D + co * P:2 * D + (co + 1) * P, :])
                    for half in range(2):
                        Mp = pM.tile([P, 2 * TCH], f32, name="Mp", tag="M")
                        for t2 in range(2):
                            tch = 2 * half + t2
                            for k in range(KT):
                                nc.tensor.matmul(
                                    Mp[:, t2 * TCH:(t2 + 1) * TCH],
                                    lhsT=w_tiles[k][:, co * P:(co + 1) * P],
                                    rhs=A_tiles[k][:, tch * TCH:(tch + 1) * TCH],
                                    start=(k == 0),
                                    stop=(k == KT - 1),
                                )
                        sl = slice(half * 2 * TCH, (half + 1) * 2 * TCH)
                        nc.vector.tensor_tensor(
                            out=out0[:, sl], in0=Mp, in1=Rb[:, sl], op=Alu.mult,
                        )
                        nc.vector.tensor_scalar_add(out0[:, sl], out0[:, sl], mean_tiles[co])
                        nc.vector.tensor_sub(out2[:, sl], x2_c[:, sl], out0[:, sl])
                    if not paired_stores:
                        nc.gpsimd.dma_start(out=out[b, co * P:(co + 1) * P, :], in_=out0)
                        nc.gpsimd.dma_start(
                            out=out[b, 2 * D + co * P:2 * D + (co + 1) * P, :], in_=out2,
                        )
                if paired_stores:
                    nc.gpsimd.dma_start(
                        out=out[b, ci * 2 * P:(ci + 1) * 2 * P, :].rearrange(
                            "(j p) t -> p j t", p=P), in_=o0p,
                    )
                    nc.gpsimd.dma_start(
                        out=out[b, 2 * D + ci * 2 * P:2 * D + (ci + 1) * 2 * P, :].rearrange(
                            "(j p) t -> p j t", p=P), in_=o2p,
                    )
    nc.compile()
    return nc


def _get_nc(loop_iters=None):
    key = ("nc", loop_iters)
    if key not in _cache:
        _cache[key] = _build_nc(loop_iters)
    return _cache[key]


def kernel(x: np.ndarray, W: np.ndarray) -> np.ndarray:
    from concourse.bass_utils import run_bass_kernel_spmd

    x = np.ascontiguousarray(x, dtype=np.float32)
    W = np.ascontiguousarray(W, dtype=np.float32)
    assert x.shape == (NCORES * NB, 3 * D, T) and W.shape == (D, D)

    nc = _get_nc()
    in_maps = [
        {"x": x[i * NB:(i + 1) * NB], "W": W} for i in range(NCORES)
    ]
    res = run_bass_kernel_spmd(nc, in_maps, core_ids=list(range(NCORES)))
    return np.concatenate([r["out"] for r in res.results], axis=0)

